# revision 1
# baseline (speedup 1.0000x reference)
"""Trainium2 Bass kernel for nn_BaseConv_137438953680.

Computation (per reference):
  h  = silu(causal_dwconv(u, w1, b1))       # k=3 depthwise
  v  = causal_dwconv(h, w2, b2)             # k=128 depthwise
  p  = silu(u @ Wp.T + bp)                  # square projection
  y  = v * p

Sharding: data-parallel over (batch, half-length) -> 8 chunks of 2048
timesteps, one per NeuronCore. Causal halo (256 steps) is materialized
host-side (zero-padded at batch starts). No collectives.

Per-core mapping:
  - conv1: channel-major on VectorE from host-transposed uT (shifts = free-axis
    offsets, per-channel weights = per-partition scalars), SiLU on ScalarE.
  - h transposed to time-major via TensorE tile transposes.
  - conv2: overlap-save spectral method. 256-pt real DFT as matmuls with
    shared (host-precomputed) DFT matrices; per-channel spectral multiply on
    VectorE; inverse DFT as matmuls. Turns the depthwise conv into
    shared-weight matmuls, avoiding per-channel weight reloads on the PE.
  - GEMM u @ Wp.T: TensorE, lhsT = uT tiles, rhs = host-pretransposed WpT,
    bias via a rank-1 (K=1) accumulating matmul, SiLU+PSUM-drain on ScalarE.
  - final elementwise multiply on VectorE.
"""
import sys

sys.path.insert(0, "/opt/trn_rl_repo")

import numpy as np
import concourse.bass as bass
import concourse.mybir as mybir
import concourse.bacc as bacc
import concourse.tile as tile
from concourse.bass_utils import run_bass_kernel_spmd

B, L, D = 4, 4096, 1024
NCORES = 8
HOP = 128
NFFT = 256
HALO = 256          # u halo steps (>= 130 needed; 2 full tiles)
NB_FULL = 16        # output blocks of 128 per core (16*128 = 2048)
KD = D // 128       # 8 d-tiles

# Matmul dtype for DFT/IDFT/GEMM: float32 (exact, 4 cyc/row) or float32r
# (~1.5e-4 scale-relative error, 1 cyc/row at free>=256).
MM_DT = mybir.dt.float32

_nc_cache: dict = {}


# ---------------------------------------------------------------- host consts
def _dft_consts():
    """Forward/inverse real-DFT matrices, packed for SBUF tiles (f32)."""
    s = np.arange(NFFT)
    F = np.zeros((NFFT, NFFT))  # [sample, row] rows: 0..128 Re, 129..255 Im
    for k in range(129):
        F[:, k] = np.cos(2 * np.pi * k * s / NFFT)
    for k in range(1, 128):
        F[:, 128 + k] = -np.sin(2 * np.pi * k * s / NFFT)
    M = np.zeros((NFFT, HOP))  # [row, m-128]
    for mi in range(HOP):
        m = 128 + mi
        M[0, mi] = 1.0 / NFFT
        M[128, mi] = ((-1) ** m) / NFFT
        for k in range(1, 128):
            M[k, mi] = 2.0 * np.cos(2 * np.pi * k * m / NFFT) / NFFT
            M[128 + k, mi] = -2.0 * np.sin(2 * np.pi * k * m / NFFT) / NFFT
    # Pack: Fm_pack[p, (st*2+bt)*128 + m] = F[st*128+p, bt*128+m]
    Fm = np.zeros((128, 512), dtype=np.float32)
    for st in range(2):
        for bt in range(2):
            Fm[:, (st * 2 + bt) * 128:(st * 2 + bt + 1) * 128] = \
                F[st * 128:(st + 1) * 128, bt * 128:(bt + 1) * 128]
    Mi = np.zeros((128, 256), dtype=np.float32)
    for kt in range(2):
        Mi[:, kt * 128:(kt + 1) * 128] = M[kt * 128:(kt + 1) * 128, :]
    return Fm, Mi


def _spectral_weights(w2):
    """Pointwise coefficient tiles C0..C3, each [128, D] (see mock.py)."""
    d = w2.shape[1]
    f = np.zeros((NFFT, d))
    f[:128] = w2[::-1, :]
    k = np.arange(NFFT)[:, None]
    n = np.arange(NFFT)[None, :]
    W = np.exp(-2j * np.pi * k * n / NFFT)
    Fh = W @ f
    Fr, Fi = Fh.real, Fh.imag
    C0 = Fr[0:128].copy()
    C1 = np.zeros((128, d)); C1[1:] = -Fi[1:128]
    C2 = np.empty((128, d)); C2[0] = Fr[128]; C2[1:] = Fr[1:128]
    C3 = np.zeros((128, d)); C3[1:] = Fi[1:128]
    return np.concatenate([C0, C1, C2, C3], axis=1).astype(np.float32)  # [128, 4*D]


def host_consts(w1, b1, w2, b2, Wp, bp):
    w1r = np.asarray(w1, np.float64)[:, 0, :]   # (3, D)
    w2r = np.asarray(w2, np.float64)[:, 0, :]   # (128, D)
    Fm, Mi = _dft_consts()
    Cs = _spectral_weights(w2r)
    # per-k-tile per-partition scalars for conv1
    w1s = np.zeros((128, 3 * KD), dtype=np.float32)
    b1s = np.zeros((128, KD), dtype=np.float32)
    for k in range(KD):
        for j in range(3):
            w1s[:, j * KD + k] = w1r[j, k * 128:(k + 1) * 128]
        b1s[:, k] = np.asarray(b1, np.float64)[k * 128:(k + 1) * 128]
    WpT = np.ascontiguousarray(np.asarray(Wp, np.float32).T)      # [D, D]
    b2r = (NFFT * np.asarray(b2, np.float64)).astype(np.float32)[None, :]  # [1, D]
    bp1 = np.asarray(bp, np.float32)[None, :]                     # [1, D]
    eye = np.eye(128, dtype=np.float32)
    return dict(Fm=Fm, Minv=Mi, Cs=Cs, w1s=w1s, b1s=b1s, WpT=WpT,
                b2r=b2r, bp1=bp1, eye=eye)


def core_chunks(u):
    """Split u (B, L, D) into NCORES host chunks: uT [D, HALO+T] + hmask."""
    u = np.asarray(u)
    T = (B * L) // NCORES          # 2048
    chunks = []
    for ci in range(NCORES):
        bi, half = divmod(ci, NCORES // B)
        t0 = half * T
        chunk = np.zeros((HALO + T, D), dtype=np.float32)
        lo = max(0, t0 - HALO)
        chunk[HALO - (t0 - lo):] = u[bi, lo:t0 + T]
        uT = np.ascontiguousarray(chunk.T)          # [D, HALO+T]
        hmask = np.full((128, 1), 0.0 if half == 0 else 1.0, dtype=np.float32)
        chunks.append((uT, hmask))
    return chunks


# ---------------------------------------------------------------- bass build
def build_nc(n_blocks=NB_FULL, mm_dt=MM_DT, reps=1):
    T = n_blocks * HOP
    W = HALO + T                       # uT width
    nc = bacc.Bacc("TRN2", target_bir_lowering=False, debug=False)
    f32 = mybir.dt.float32

    uT_d = nc.dram_tensor("uT", [D, W], f32, kind="ExternalInput").ap()
    WpT_d = nc.dram_tensor("WpT", [D, D], f32, kind="ExternalInput").ap()
    Fm_d = nc.dram_tensor("Fm", [128, 512], f32, kind="ExternalInput").ap()
    Mi_d = nc.dram_tensor("Minv", [128, 256], f32, kind="ExternalInput").ap()
    Cs_d = nc.dram_tensor("Cs", [128, 4 * D], f32, kind="ExternalInput").ap()
    w1s_d = nc.dram_tensor("w1s", [128, 3 * KD], f32, kind="ExternalInput").ap()
    b1s_d = nc.dram_tensor("b1s", [128, KD], f32, kind="ExternalInput").ap()
    b2r_d = nc.dram_tensor("b2r", [1, D], f32, kind="ExternalInput").ap()
    bp1_d = nc.dram_tensor("bp1", [1, D], f32, kind="ExternalInput").ap()
    eye_d = nc.dram_tensor("eye", [128, 128], f32, kind="ExternalInput").ap()
    hm_d = nc.dram_tensor("hmask", [128, 1], f32, kind="ExternalInput").ap()
    y_d = nc.dram_tensor("y", [T, D], f32, kind="ExternalOutput").ap()

    uT3 = uT_d.rearrange("(k p) t -> p k t", p=128)
    WpT3 = WpT_d.rearrange("(k p) e -> p k e", p=128)

    from contextlib import ExitStack
    with tile.TileContext(nc) as tc, ExitStack() as ctx:
        cpool = ctx.enter_context(tc.tile_pool(name="consts", bufs=1))
        # resident constants
        wpt = cpool.tile([128, KD * D], f32, tag="wpt")
        nc.sync.dma_start(wpt[:].rearrange("p (k e) -> p k e", k=KD), WpT3)
        fm = cpool.tile([128, 512], f32, tag="fm")
        nc.sync.dma_start(fm[:], Fm_d[:])
        mi = cpool.tile([128, 256], f32, tag="mi")
        nc.sync.dma_start(mi[:], Mi_d[:])
        cs = cpool.tile([128, 4 * D], f32, tag="cs")
        nc.sync.dma_start(cs[:], Cs_d[:])
        w1s = cpool.tile([128, 3 * KD], f32, tag="w1s")
        nc.sync.dma_start(w1s[:], w1s_d[:])
        b1s = cpool.tile([128, KD], f32, tag="b1s")
        nc.sync.dma_start(b1s[:], b1s_d[:])
        b2r = cpool.tile([1, D], f32, tag="b2r")
        nc.sync.dma_start(b2r[:], b2r_d[:])
        bp1 = cpool.tile([1, D], f32, tag="bp1")
        nc.sync.dma_start(bp1[:], bp1_d[:])
        eye = cpool.tile([128, 128], f32, tag="eye")
        nc.sync.dma_start(eye[:], eye_d[:])
        hm = cpool.tile([128, 1], f32, tag="hm")
        nc.sync.dma_start(hm[:], hm_d[:])
        ones1 = cpool.tile([1, 128], f32, tag="ones1")
        nc.gpsimd.memset(ones1[:], 1.0)

        upool = ctx.enter_context(tc.tile_pool(name="uq", bufs=3))
        scr = ctx.enter_context(tc.tile_pool(name="scr", bufs=6))
        hcm_p = ctx.enter_context(tc.tile_pool(name="hcm", bufs=2))
        hsb_p = ctx.enter_context(tc.tile_pool(name="hsb", bufs=3))
        yt_p = ctx.enter_context(tc.tile_pool(name="yt", bufs=4))
        psb_p = ctx.enter_context(tc.tile_pool(name="psb", bufs=4))
        ysb_p = ctx.enter_context(tc.tile_pool(name="ysb", bufs=2))

        htr_p = ctx.enter_context(tc.tile_pool(name="htr", bufs=1, space="PSUM"))
        xps_p = ctx.enter_context(tc.tile_pool(name="xps", bufs=1, space="PSUM"))
        vps_p = ctx.enter_context(tc.tile_pool(name="vps", bufs=2, space="PSUM"))
        pps_p = ctx.enter_context(tc.tile_pool(name="pps", bufs=2, space="PSUM"))

        MULT = mybir.AluOpType.mult
        ADD = mybir.AluOpType.add
        SILU = mybir.ActivationFunctionType.Silu

        def mk_h_tile(hq):
            """conv1 (c-major, DVE+GPS) + silu (ACT) + transpose (PE) to a
            time-major h tile [128(t), D(ch)]."""
            base = HALO + hq * HOP
            uq = upool.tile([128, KD, 130], f32, tag="uq")
            nc.sync.dma_start(uq[:], uT3[:, :, base - 2:base + 128])
            hcm = hcm_p.tile([128, KD * 128], f32, tag="hcm")
            for k in range(KD):
                t1 = scr.tile([128, 128], f32, tag="scr1")
                nc.gpsimd.tensor_scalar(
                    t1[:], uq[:, k, 0:128], w1s[:, 0 * KD + k:0 * KD + k + 1],
                    None, MULT)
                t2 = scr.tile([128, 128], f32, tag="scr2")
                nc.gpsimd.tensor_scalar(
                    t2[:], uq[:, k, 1:129], w1s[:, 1 * KD + k:1 * KD + k + 1],
                    None, MULT)
                t3 = scr.tile([128, 128], f32, tag="scr3")
                nc.gpsimd.tensor_tensor(t3[:], t1[:], t2[:], ADD)
                t4 = scr.tile([128, 128], f32, tag="scr4")
                nc.vector.tensor_scalar(
                    t4[:], uq[:, k, 2:130], w1s[:, 2 * KD + k:2 * KD + k + 1],
                    b1s[:, k:k + 1], MULT, ADD)
                nc.vector.tensor_tensor(
                    hcm[:, k * 128:(k + 1) * 128], t3[:], t4[:], ADD)
            hcm2 = hcm_p.tile([128, KD * 128], f32, tag="hcm2")
            nc.scalar.activation(hcm2[:], hcm[:], SILU)
            htr = htr_p.tile([128, D], f32, tag="htr")
            for k in range(KD):
                nc.tensor.transpose(
                    htr[:, k * 128:(k + 1) * 128],
                    hcm2[:, k * 128:(k + 1) * 128], eye[:])
            hsb = hsb_p.tile([128, D], f32, tag="hsb")
            if hq < 0:
                nc.vector.tensor_scalar_mul(hsb[:], htr[:], hm[:, 0:1])
            else:
                nc.vector.tensor_copy(hsb[:], htr[:])
            return uq, hsb

        from contextlib import nullcontext
        loop_ctx = tc.For_i(0, reps, 1) if reps > 1 else nullcontext()
        with loop_ctx:
            h_tiles: dict = {}
            uq_tiles: dict = {}
            uq_tiles[-1], h_tiles[-1] = mk_h_tile(-1)
            uq_tiles[0], h_tiles[0] = mk_h_tile(0)
            for q in range(n_blocks):
                uq = uq_tiles.pop(q)
                hsb = h_tiles[q]
                hprev = h_tiles.pop(q - 1)
                ysb = ysb_p.tile([128, D], f32, tag="ysb")
                # ---- GEMM both halves (PE work first; only needs uq + consts)
                pps_t = []
                for half in range(2):
                    e0 = half * 512
                    pps = pps_p.tile([128, 512], f32, tag="pps")
                    for k in range(KD):
                        nc.tensor.matmul(
                            pps[:],
                            uq[:, k, 2:130].bitcast(mm_dt),
                            wpt[:, k * D + e0:k * D + e0 + 512].bitcast(mm_dt),
                            start=(k == 0), stop=False)
                    nc.tensor.matmul(
                        pps[:], ones1[:].bitcast(mm_dt),
                        bp1[:, e0:e0 + 512].bitcast(mm_dt),
                        start=False, stop=True)
                    pps_t.append(pps)
                # ---- forward DFT both halves
                x_t = []
                for half in range(2):
                    e0 = half * 512
                    x0 = xps_p.tile([128, 512], f32, tag="xps0")
                    x1 = xps_p.tile([128, 512], f32, tag="xps1")
                    for bt, xps in ((0, x0), (1, x1)):
                        nc.tensor.matmul(
                            xps[:],
                            fm[:, (0 * 2 + bt) * 128:(0 * 2 + bt + 1) * 128].bitcast(mm_dt),
                            hprev[:, e0:e0 + 512].bitcast(mm_dt),
                            start=True, stop=False)
                        nc.tensor.matmul(
                            xps[:],
                            fm[:, (1 * 2 + bt) * 128:(1 * 2 + bt + 1) * 128].bitcast(mm_dt),
                            hsb[:, e0:e0 + 512].bitcast(mm_dt),
                            start=False, stop=True)
                    x_t.append((x0, x1))
                # ---- silu(p) early: frees GEMM PSUM banks a block sooner
                psb_t = []
                for half in range(2):
                    psb = psb_p.tile([128, 512], f32, tag="psb")
                    nc.scalar.activation(psb[:], pps_t[half][:], SILU)
                    psb_t.append(psb)
                # ---- spectral pointwise (DVE muls read PSUM; GPS does adds)
                yt_t = []
                for half in range(2):
                    e0 = half * 512
                    x0, x1 = x_t[half]
                    yt0 = yt_p.tile([128, 512], f32, tag="yt0")
                    yt1 = yt_p.tile([128, 512], f32, tag="yt1")
                    ta = scr.tile([128, 512], f32, tag="scra")
                    tb = scr.tile([128, 512], f32, tag="scrb")
                    nc.vector.tensor_tensor(yt0[:], x0[:], cs[:, 0 * D + e0:0 * D + e0 + 512], MULT)
                    nc.vector.tensor_tensor(ta[:], x1[:], cs[:, 1 * D + e0:1 * D + e0 + 512], MULT)
                    nc.gpsimd.tensor_tensor(yt0[:], yt0[:], ta[:], ADD)
                    nc.vector.tensor_tensor(
                        yt0[0:1, :], yt0[0:1, :], b2r[0:1, e0:e0 + 512], ADD)
                    nc.vector.tensor_tensor(yt1[:], x1[:], cs[:, 2 * D + e0:2 * D + e0 + 512], MULT)
                    nc.vector.tensor_tensor(tb[:], x0[:], cs[:, 3 * D + e0:3 * D + e0 + 512], MULT)
                    nc.gpsimd.tensor_tensor(yt1[:], yt1[:], tb[:], ADD)
                    yt_t.append((yt0, yt1))
                # ---- next block's h (PE transposes slot between DFT and IDFT,
                #      giving DVE/GPS time to finish pointwise)
                if q + 1 < n_blocks:
                    uq_tiles[q + 1], h_tiles[q + 1] = mk_h_tile(q + 1)
                # ---- inverse DFT + silu(p) + final multiply
                for half in range(2):
                    e0 = half * 512
                    yt0, yt1 = yt_t[half]
                    vps = vps_p.tile([128, 512], f32, tag="vps")
                    nc.tensor.matmul(vps[:], mi[:, 0:128].bitcast(mm_dt),
                                     yt0[:].bitcast(mm_dt), start=True, stop=False)
                    nc.tensor.matmul(vps[:], mi[:, 128:256].bitcast(mm_dt),
                                     yt1[:].bitcast(mm_dt), start=False, stop=True)
                    nc.vector.tensor_tensor(
                        ysb[:, e0:e0 + 512], vps[:], psb_t[half][:], MULT)
                nc.sync.dma_start(y_d[q * HOP:(q + 1) * HOP, :], ysb[:])

    nc.compile()
    return nc


def get_nc(n_blocks=NB_FULL, mm_dt=MM_DT, reps=1):
    key = (n_blocks, str(mm_dt), reps)
    if key not in _nc_cache:
        _nc_cache[key] = build_nc(n_blocks, mm_dt, reps)
    return _nc_cache[key]


# ---------------------------------------------------------------- entry point
def kernel(u, w1, b1, w2, b2, Wp, bp):
    u = np.asarray(u, dtype=np.float32)
    consts = host_consts(w1, b1, w2, b2, Wp, bp)
    chunks = core_chunks(u)
    in_maps = []
    for (uT, hmask) in chunks:
        m = dict(consts)
        m["uT"] = uT
        m["hmask"] = hmask
        in_maps.append(m)
    nc = get_nc()
    res = run_bass_kernel_spmd(nc, in_maps, core_ids=list(range(NCORES)))
    T = (B * L) // NCORES
    y = np.empty((B, L, D), dtype=np.float32)
    for ci in range(NCORES):
        bi, half = divmod(ci, NCORES // B)
        y[bi, half * T:(half + 1) * T] = res.results[ci]["y"]
    return y

